# revision 13
# baseline (speedup 1.0000x reference)
"""Trainium2 Bass kernel for the masked per-site stencil contraction

    y[o, n] = f( sum_{i,k} Wconv[o,i,k] * mask[n,o,i,k] * x[i, shifts[n,k]] + bconv[o] )
    f(v) = (sigmoid(v) - 0.5) * (2 + 2e)/(e - 1) = (2+2e)/(2(e-1)) * tanh(v/2)

Shapes: O=I=32, K=13, N=4096.  Sharded over 8 NeuronCores along the site
dimension N (512 sites per core); mask / shifts / output columns are
partitioned, x / Wconv / bconv replicated.

Per-core device plan (all cores run the identical SPMD program):
  * layout: partition dim = (k, i) rows of the 416-long stencil axis
    (k-major, p = k*32 + i), free dim = local sites n (512).
    Chunks: c=0..2 cover k in [4c, 4c+4) -> 128 partitions each;
    the k=12 remainder is packed 4-output-channels-per-tile.
  * gather g[p, n] = x[i(p), shifts[n, k(p)]] with GPSIMD ap_gather
    (x replicated to 128 partitions; indices pre-wrapped host-side).
  * DVE: prod = mask_tile * g  (the only full-size elementwise pass)
  * PE:  y[o, n] = sum_p W[o, p] * prod_o[p, n] as a 4-chunk accumulated
    matvec per output channel, lhsT = W column, float32r (1 cyc/row).
  * ACT: y = tanh(0.5*y + 0.5*b), DVE: * scale/2; DMA out.
"""

import math

import numpy as np

import concourse.bacc as bacc
import concourse.mybir as mybir
from concourse import tile
from concourse.bass_utils import run_bass_kernel_spmd

O, I, K, N = 32, 32, 13, 4096
NCORES = 8
NS = N // NCORES          # 512 local sites per core
IK = K * I                # 416 stencil rows, k-major: p = k*32 + i
BIG = 12 * I              # 384 rows in the three 128-partition chunks
_E = math.e
SCALE = (2.0 + 2.0 * _E) / (_E - 1.0)

_F32 = mybir.dt.float32
_F32R = mybir.dt.float32r
_I16 = mybir.dt.int16

_BUILT = {}


def _build():
    """Build + compile the SPMD Bass program once per process."""
    if "nc" in _BUILT:
        return _BUILT["nc"]

    nc = bacc.Bacc("TRN2", target_bir_lowering=False, debug=False)

    d_x4 = nc.declare_dram_parameter("x4", [128, N], _F32, isOutput=False)
    d_wt = nc.declare_dram_parameter("wt", [IK, O], _F32, isOutput=False)
    d_b = nc.declare_dram_parameter("brow", [1, O], _F32, isOutput=False)
    d_mask = nc.declare_dram_parameter("maskt", [O, IK, NS], _F32, isOutput=False)
    d_idxb = nc.declare_dram_parameter("idxb", [128, 96], _I16, isOutput=False)
    d_idx3 = nc.declare_dram_parameter("idx3", [128, 32], _I16, isOutput=False)
    d_y = nc.declare_dram_parameter("y", [O, NS], _F32, isOutput=True)

    with tile.TileContext(nc) as tc:
        with (
            tc.tile_pool(name="const", bufs=1) as cpool,
            tc.tile_pool(name="gather", bufs=1) as gpool,
            tc.tile_pool(name="mask", bufs=4) as mpool,
            tc.tile_pool(name="m3", bufs=2) as m3pool,
            tc.tile_pool(name="prod", bufs=4) as ppool,
            tc.tile_pool(name="p3", bufs=2) as p3pool,
            tc.tile_pool(name="out", bufs=1) as opool,
            tc.tile_pool(name="psum", bufs=1, space="PSUM") as qpool,
        ):
            x_sb = cpool.tile([128, N], _F32)
            nc.sync.dma_start(x_sb[:, :], d_x4[:, :])
            wt_big = cpool.tile([128, 3, O], _F32)
            nc.sync.dma_start(
                wt_big[:, :, :],
                d_wt[0:BIG, :].rearrange("(c p) m -> p c m", p=128),
            )
            # k=12 weight rows replicated into partition bases {0, 32} so the
            # c3 matmul's lhsT base matches its rhs base (matmul operand bases
            # are limited to 0/32/64).
            wt3r = cpool.tile([64, O], _F32)
            for j in range(2):
                nc.sync.dma_start(wt3r[32 * j : 32 * j + 32, :], d_wt[BIG:IK, :])
            # FP32R copies of the weights (FP32R matmul operands must be
            # produced already rounded to FP32R)
            wt_bigr = cpool.tile([128, 3, O], _F32R)
            nc.vector.tensor_copy(wt_bigr[:, :, :], wt_big[:, :, :])
            wt3rr = cpool.tile([64, O], _F32R)
            nc.vector.tensor_copy(wt3rr[:, :], wt3r[:, :])
            idxb_sb = cpool.tile([128, 96], _I16)
            nc.sync.dma_start(idxb_sb[:, :], d_idxb[:, :])
            idx3_sb = cpool.tile([128, 32], _I16)
            nc.sync.dma_start(idx3_sb[:, :], d_idx3[:, :])
            brow_sb = cpool.tile([1, O], _F32)
            nc.sync.dma_start(brow_sb[:, :], d_b[:, :])

            # g[p, c*512 + n] = x[p%32, shifts[n, 4c + p//32]]
            g_big = gpool.tile([128, 3, NS], _F32)
            nc.gpsimd.ap_gather(
                g_big[:, :, :], x_sb[:, :], idxb_sb[:, :],
                channels=128, num_elems=N, d=1, num_idxs=3 * NS,
            )
            # g3rep[p, n] = x[p%32, shifts[n, 12]]  (k=12 row, 2x replicated)
            g3rep = gpool.tile([64, NS], _F32)
            nc.gpsimd.ap_gather(
                g3rep[:, :], x_sb[0:64, :], idx3_sb[0:64, :],
                channels=64, num_elems=N, d=1, num_idxs=NS,
            )

            # (sigmoid(y + b) - 0.5) * SCALE == SCALE/2 * tanh((y + b)/2)
            bhalf = opool.tile([1, O], _F32)
            nc.scalar.activation(
                bhalf[:, :], brow_sb[:, :], mybir.ActivationFunctionType.Copy,
                scale=0.5,
            )
            ystage = opool.tile([O, NS], _F32)

            for og in range(O // 4):
                # k=12 rows for output channels 4*og .. 4*og+3, packed two per
                # 64-partition tile (matmul operand bases limited to 0/32/64)
                p3ps = []
                for h in range(2):
                    m3p = m3pool.tile([64, NS], _F32, tag=f"m3p{h}")
                    for oo in range(2):
                        nc.sync.dma_start(
                            m3p[32 * oo : 32 * oo + 32, :],
                            d_mask[4 * og + 2 * h + oo, BIG:IK, :],
                        )
                    p3p = p3pool.tile([64, NS], _F32R, tag=f"p3p{h}")
                    nc.vector.tensor_mul(p3p[:, :], m3p[:, :], g3rep[:, :])
                    p3ps.append(p3p)

                for j in range(4):
                    o = 4 * og + j
                    mt = mpool.tile([128, 3, NS], _F32, tag="mt")
                    nc.sync.dma_start(
                        mt[:, :, :],
                        d_mask[o, 0:BIG, :].rearrange("(c p) n -> p c n", p=128),
                    )
                    pt = ppool.tile([128, 3, NS], _F32R, tag="pt")
                    nc.vector.tensor_mul(pt[:, :, :], mt[:, :, :], g_big[:, :, :])
                    yp = qpool.tile([1, NS], _F32, tag="yp", bufs=4)
                    for c in range(3):
                        nc.tensor.matmul(
                            yp[:, :],
                            wt_bigr[:, c, o : o + 1],
                            pt[:, c, :],
                            start=(c == 0),
                            stop=False,
                        )
                    h, jj = divmod(j, 2)
                    nc.tensor.matmul(
                        yp[:, :],
                        wt3rr[32 * jj : 32 * jj + 32, o : o + 1],
                        p3ps[h][32 * jj : 32 * jj + 32, :],
                        start=False,
                        stop=True,
                    )
                    ytmp = opool.tile([1, NS], _F32, tag="ytmp", bufs=4)
                    nc.scalar.activation(
                        ytmp[:, :], yp[:, :], mybir.ActivationFunctionType.Tanh,
                        bias=bhalf[0:1, o : o + 1], scale=0.5,
                    )
                    nc.sync.dma_start(ystage[o : o + 1, :], ytmp[:, :])

            nc.vector.tensor_scalar_mul(ystage[:, :], ystage[:, :], SCALE / 2.0)
            nc.sync.dma_start(d_y[:, :], ystage[:, :])

    nc.compile()
    _BUILT["nc"] = nc
    return nc


def _wrap16(col):
    """shifts column (NS,) -> (16, NS//16) wrapped layout: out[r, s] = col[s*16+r]."""
    return np.ascontiguousarray(col.reshape(NS // 16, 16).T)


def make_in_maps(x, Wconv, bconv, mask, shifts):
    """Host-side shard/layout prep. Pure data movement (+ dtype-preserving
    int32->int16 index narrowing; indices are < 4096)."""
    x = np.ascontiguousarray(x, dtype=np.float32)
    x4 = np.ascontiguousarray(np.tile(x, (4, 1)))                   # (128, N)
    wt = np.ascontiguousarray(
        Wconv.astype(np.float32, copy=False).transpose(2, 1, 0)
    ).reshape(IK, O)                                                # (416, 32)
    brow = np.ascontiguousarray(
        bconv.astype(np.float32, copy=False).reshape(1, O)
    )
    mask = np.asarray(mask, dtype=np.float32)
    shifts = np.asarray(shifts)

    in_maps = []
    for core in range(NCORES):
        sl = slice(core * NS, (core + 1) * NS)
        maskt = np.ascontiguousarray(
            mask[sl].transpose(1, 3, 2, 0)
        ).reshape(O, IK, NS)                                        # (O, 416, NS)
        sh = shifts[sl].astype(np.int16)                            # (NS, 13)
        idxb = np.empty((128, 96), np.int16)
        for g in range(8):
            for c in range(3):
                idxb[16 * g : 16 * g + 16, 32 * c : 32 * c + 32] = _wrap16(
                    sh[:, 4 * c + g // 2]
                )
        w12 = _wrap16(sh[:, 12])
        idx3 = np.empty((128, 32), np.int16)
        for g in range(8):
            idx3[16 * g : 16 * g + 16, :] = w12
        in_maps.append(
            {
                "x4": x4,
                "wt": wt,
                "brow": brow,
                "maskt": maskt,
                "idxb": idxb,
                "idx3": idx3,
            }
        )
    return in_maps


def kernel(x, Wconv, bconv, mask, shifts):
    nc = _build()
    in_maps = make_in_maps(x, Wconv, bconv, mask, shifts)
    res = run_bass_kernel_spmd(nc, in_maps, core_ids=list(range(NCORES)))
    y = np.empty((O, N), np.float32)
    for core in range(NCORES):
        y[:, core * NS : (core + 1) * NS] = res.results[core]["y"]
    return y
